# revision 1
# baseline (speedup 1.0000x reference)
"""ApproxRepSet kernel for 8 TRN2 NeuronCores.

reference:
  t = relu(X @ Wc)            # [B, P, H*E], k = e*H + h
  t = max over e              # [B, P, H]
  t = sum over p              # [B, H]
  t = relu(t @ w1 + b1); t = relu(t @ w2 + b2); out = t @ w3 + b3

Sharding: data-parallel over batch, 16 batches per core. Weights replicated.

Per-core layout (host-side, zero on-device transposes):
  - X shard [16*1024, 64] packed as A[128, 8192]: partition 64*(r%2)+d,
    free r//2.  A 256-row block i lives at free cols [128i, 128i+128): even
    rows on partitions 0:64, odd rows on 64:128.  Each half is the matmul
    stationary lhsT [K=64, M=128]; the two halves run concurrently via PE
    row tiling (tile_position (0,0)/(64,0)).
  - X/Wc cast to bf16 on host; Wc columns reordered k' = h*16 + e so the
    max over e is an innermost free-dim window; Wc stacked twice on
    partitions for row tiling.

Pooling (the throughput wall: every Y element must leave PSUM through DVE
at 0.96 G/lane or ACT at 1.2 G/lane, 1 elem/lane/cycle, both 1x-capped for
fp32 PSUM reads; GPSIMD/DMA have no PSUM port and the ISA rejects
TensorTensor on the Pool engine, so these two engines are all there is):
  - Blocks processed in supers of 16 (4 batches), roles a:b = 5:11:
    path a: DVE tensor_reduce(max) straight from PSUM   (~1.2us/blk DVE)
    path b: ACT relu-cast PSUM->SBUF bf16               (~1.1us/blk ACT)
  - The 11 b-blocks of a super share one binary TT-max tree (bf16 SBUF
    runs in 2x DVE mode; wide trees amortize the ~151c fixed cost per op).
    The last level is scalar_tensor_tensor (out = (u0 max 0) max u1)
    fusing the relu.  Trees run one super deferred, split 6+5 around the
    current super's blocks so the DVE queue never blocks the PSUM
    rotation; the final super's tree is split 4+5+2 so only a 2-block
    tree is exposed after the last ACT move.
  - a:b = 5:11 balances DVE (reduces + trees) against ACT (moves); both
    end up ~95% busy, which is the two-engine drain floor for the
    64 blocks x 1024 fp32/lane that must exit PSUM.
  - Sum over p: ones-vector matmuls (lhsT = mb [128, 32] slices,
    rhs = ones [128,1]) accumulating S^T [32, 16] in PSUM, deferred two
    supers so PE never stalls the pipeline on tree results.
  - MLP stays transposed end-to-end: h1^T = w1^T @ S^T etc., with
    relu(x+bias) as DVE scalar_tensor_tensor against a zeros tile.
  - Startup: block 0's stationary columns + wc DMA'd first; a dummy
    ACTIVATE hoists the ~2.7us Relu table load into the DMA wait.
"""

import sys

import numpy as np

sys.path.insert(0, "/opt/trn_rl_repo")

import ml_dtypes
import concourse.bass as bass
import concourse.mybir as mybir
import concourse.tile as tile
from concourse import bacc
from concourse.bass_utils import run_bass_kernel_spmd

B, P, D = 128, 1024, 64
H, E = 32, 16
HE = H * E  # 512
NOUT = 10
NCORES = 8
BPC = B // NCORES  # 16 batches per core
R = BPC * P  # 16384 rows per core
NBLK = R // 256  # 64 blocks of 256 rows
NGRP = NBLK // 8  # 8 groups of 8 blocks (2 batches each)
FCHUNK = 2048  # free-dim cols per DMA chunk (= 16 blocks)

FP32 = mybir.dt.float32
BF16 = mybir.dt.bfloat16
AX = mybir.AxisListType
ALU = mybir.AluOpType
ACT_F = mybir.ActivationFunctionType

_cache = {}


def _build_nc():
    nc = bacc.Bacc(
        "TRN2", target_bir_lowering=False, debug=False, num_devices=NCORES
    )

    xa = nc.declare_dram_parameter("xa", [128, R // 2], BF16, isOutput=False)
    wc = nc.declare_dram_parameter("wc", [128, HE], BF16, isOutput=False)
    # packed MLP weights [65, 138] f32 with biases folded in as extra rows:
    # [w1;b1] rows 0:33 cols 0:64, [w2;b2] rows 0:65 cols 64:128,
    # [w3;b3] rows 0:65 cols 128:138
    wmlp = nc.declare_dram_parameter("wmlp", [65, 138], FP32, isOutput=False)
    out = nc.declare_dram_parameter("out", [NOUT, BPC], FP32, isOutput=True)

    with tile.TileContext(nc) as tc:
        with (
            tc.tile_pool(name="const", bufs=1) as const_pool,
            tc.tile_pool(name="xa", bufs=2) as xa_pool,
            tc.tile_pool(name="mb", bufs=3) as mb_pool,
            tc.tile_pool(name="yb", bufs=2) as yb_pool,
            tc.tile_pool(name="tree", bufs=1) as tree_pool,
            tc.tile_pool(name="mlp", bufs=1) as mlp_pool,
            tc.tile_pool(name="ypsum", bufs=3, space=bass.MemorySpace.PSUM) as ypsum_pool,
            tc.tile_pool(name="spsum", bufs=1, space=bass.MemorySpace.PSUM) as spsum_pool,
        ):
            # --- persistent tiles; wc + first xa chunk first (critical path) ---
            xa_tiles = []
            for c in range(4):
                t = xa_pool.tile([128, FCHUNK], BF16, tag="xa", name="xa_sb")
                xa_tiles.append(t)
            # block 0's stationary columns first (LDWEIGHTS can start before
            # wc lands), then wc (the moving operand), then the rest
            wc_sb = const_pool.tile([128, HE], BF16)
            nc.sync.dma_start(xa_tiles[0][:, 0:256], xa[:, 0:256])
            nc.sync.dma_start(wc_sb[:], wc[:])
            for lo, hi in ((256, 1024), (1024, 2048)):
                nc.sync.dma_start(xa_tiles[0][:, lo:hi], xa[:, lo:hi])
            ones_sb = const_pool.tile([128, 1], BF16)
            nc.vector.memset(ones_sb[:], 1.0)
            # MLP activations carry a trailing ones-row for the folded biases
            s_sb = const_pool.tile([H + 1, BPC], FP32)
            nc.vector.memset(s_sb[:], 1.0)
            h1_sb = const_pool.tile([65, BPC], FP32)
            nc.vector.memset(h1_sb[:], 1.0)
            h2_sb = const_pool.tile([65, BPC], FP32)
            nc.vector.memset(h2_sb[:], 1.0)
            # dummy ACTIVATE: hoists the Relu table load into the DMA-wait
            scratch_sb = const_pool.tile([128, 1], BF16)
            nc.scalar.activation(scratch_sb[:], ones_sb[:], ACT_F.Relu)

            # one PSUM bank shared by the S^T accumulator and the MLP matmuls
            sm_psum = spsum_pool.tile([64, 512], FP32)
            s_psum = sm_psum[0:H, 0:BPC]  # S^T accumulator

            def do_block(blk, role, mb, aslot, yb, bslot):
                """One 256-row block: 2 row-tiled matmuls + drain (a or b)."""
                xa_sb = xa_tiles[blk // (FCHUNK // 128)]
                f0 = (blk % (FCHUNK // 128)) * 128
                y_ps = ypsum_pool.tile([128, 2 * HE], FP32, tag="y_ps", name="y_ps")
                nc.tensor.matmul(
                    y_ps[:, 0:HE],
                    xa_sb[0:64, f0 : f0 + 128],
                    wc_sb[0:64, :],
                    start=True,
                    stop=True,
                )
                nc.tensor.matmul(
                    y_ps[:, HE : 2 * HE],
                    xa_sb[64:128, f0 : f0 + 128],
                    wc_sb[64:128, :],
                    start=True,
                    stop=True,
                )
                if role == "a":
                    # max over e=16 windows straight out of PSUM (1x DVE)
                    nc.vector.tensor_reduce(
                        mb[:, 2 * aslot : 2 * aslot + 2, :],
                        y_ps[:].rearrange("p (t h e) -> p t h e", t=2, h=H, e=E),
                        axis=AX.X,
                        op=ALU.max,
                    )
                else:
                    # relu-cast to bf16 (1x ACT); tree later
                    nc.scalar.activation(
                        yb[:, 2 * bslot : 2 * bslot + 2, :, :].rearrange(
                            "p a b c -> p (a b c)"
                        ),
                        y_ps[:],
                        ACT_F.Relu,
                    )

            def do_tree(yb, mb, s0=0, ns=11, joff=10):
                """Binary max tree over b-slots [s0, s0+ns) of yb
                [128, 22, H, 16] -> mb[:, joff+2*s0 : joff+2*(s0+ns), :],
                relu fused in the last level."""
                q0, q1 = 2 * s0, 2 * (s0 + ns)
                nq = q1 - q0
                t1 = tree_pool.tile([128, nq, H, 8], BF16, tag=f"t1_{nq}", name="t1")
                nc.vector.tensor_tensor(
                    t1[:], yb[:, q0:q1, :, 0:8], yb[:, q0:q1, :, 8:16], op=ALU.max
                )
                t2 = tree_pool.tile([128, nq, H, 4], BF16, tag=f"t2_{nq}", name="t2")
                nc.vector.tensor_tensor(
                    t2[:], t1[:, :, :, 0:4], t1[:, :, :, 4:8], op=ALU.max
                )
                t3 = tree_pool.tile([128, nq, H, 2], BF16, tag=f"t3_{nq}", name="t3")
                nc.vector.tensor_tensor(
                    t3[:], t2[:, :, :, 0:2], t2[:, :, :, 2:4], op=ALU.max
                )
                # out = (u0 max 0) max u1 : final pair max + relu in one op
                nc.vector.scalar_tensor_tensor(
                    mb[:, joff + q0 : joff + q1, :],
                    t3[:, :, :, 0],
                    0.0,
                    t3[:, :, :, 1],
                    op0=ALU.max,
                    op1=ALU.max,
                )

            # j-slot ownership per batch of a super: a-j 0..9, b-j 10..31
            JMAP = (
                (0, (0, 1, 10, 11, 12, 13, 14, 15)),
                (1, (2, 3, 16, 17, 18, 19, 20, 21)),
                (2, (4, 5, 22, 23, 24, 25, 26, 27)),
                (3, (6, 7, 8, 9, 28, 29, 30, 31)),
            )

            def finish_super(s, mb):
                """S accumulation for super s (mb fully relu'd by now)."""
                for bi, js in JMAP:
                    bidx = 4 * s + bi
                    for n, j in enumerate(js):
                        nc.tensor.matmul(
                            s_psum[:, bidx : bidx + 1],
                            mb[:, j, :],
                            ones_sb[:],
                            start=(n == 0),
                            stop=(n == 7),
                        )

            # supers of 16 blocks (4 batches): roles a:b = 5:11; the 11
            # b-blocks share one tree, amortizing the per-op fixed cost
            ROLES = (("a", 0), ("b", 0), ("b", 1), ("b", 2),
                     ("a", 1), ("b", 3), ("b", 4), ("b", 5),
                     ("a", 2), ("b", 6), ("b", 7), ("b", 8),
                     ("a", 3), ("b", 9), ("b", 10), ("a", 4))
            NSUP = NBLK // 16  # 4
            tree_pend = []  # supers awaiting tree chunk A
            treeB_pend = []  # supers awaiting tree chunk B
            ones_pend = []  # supers with pending ones-MMs
            for s in range(NSUP):
                blk0 = 16 * s
                if s > 0:
                    nc.sync.dma_start(
                        xa_tiles[s][:], xa[:, s * FCHUNK : (s + 1) * FCHUNK]
                    )
                mb = mb_pool.tile([128, 32, H], BF16, tag="mb", name="mb")
                yb = yb_pool.tile([128, 22, H, E], BF16, tag="yb", name="yb")
                do_block(blk0, ROLES[0][0], mb, ROLES[0][1], yb, ROLES[0][1])
                if treeB_pend:
                    # chunk B after the next super's first block so boundary
                    # reduces aren't queued behind a 3us tree
                    ps, pyb, pmb = treeB_pend.pop(0)
                    do_tree(pyb, pmb, 6, 5)
                    nc.vector.tensor_scalar_max(
                        pmb[:, 0:10, :], pmb[:, 0:10, :], 0.0
                    )
                    ones_pend.append((ps, pmb))
                for i, (role, slot) in enumerate(ROLES[1:8]):
                    do_block(blk0 + 1 + i, role, mb, slot, yb, slot)
                if tree_pend:
                    # chunk A between the halves so the DVE queue never
                    # blocks this super's drains for a full tree
                    ps, pyb, pmb = tree_pend.pop(0)
                    do_tree(pyb, pmb, 0, 6)
                    treeB_pend.append((ps, pyb, pmb))
                # last super: a-blocks last so the final ACT move (gating the
                # exposed tail tree) lands ~1.2us earlier; JMAP-invariant
                back = (ROLES[8:] if s < NSUP - 1 else
                        (("a", 2), ("b", 6), ("b", 7), ("b", 8),
                         ("b", 9), ("b", 10), ("a", 3), ("a", 4)))
                for i, (role, slot) in enumerate(back):
                    do_block(blk0 + 8 + i, role, mb, slot, yb, slot)
                while len(ones_pend) > (1 if s < NSUP - 1 else 0):
                    os_, omb = ones_pend.pop(0)
                    finish_super(os_, omb)
                tree_pend.append((s, yb, mb))

            # tail: finish the second-to-last super's chunk B, then the last
            # super's tree split 4+5+2 so only a 2-block tree is exposed
            # after the final ACT move; relu early so early batches'
            # S-accumulation starts as soon as their slots are pooled
            if treeB_pend:
                ps, pyb, pmb = treeB_pend.pop(0)
                do_tree(pyb, pmb, 6, 5)
                nc.vector.tensor_scalar_max(
                    pmb[:, 0:10, :], pmb[:, 0:10, :], 0.0
                )
                ones_pend.append((ps, pmb))
            ps, pyb, pmb = tree_pend.pop(0)
            nc.vector.tensor_scalar_max(pmb[:, 0:10, :], pmb[:, 0:10, :], 0.0)
            do_tree(pyb, pmb, 0, 4)
            do_tree(pyb, pmb, 4, 5)
            do_tree(pyb, pmb, 9, 2)
            ones_pend.append((ps, pmb))
            for os_, omb in ones_pend:
                finish_super(os_, omb)

            # --- MLP tail (all transposed, biases folded into the matmuls
            # via the ones-rows); weights arrive in one late DMA ---
            wmlp_sb = const_pool.tile([65, 138], FP32)
            nc.gpsimd.dma_start(wmlp_sb[:], wmlp[:])
            w1_sb = wmlp_sb[0 : H + 1, 0:64]
            w2_sb = wmlp_sb[0:65, 64:128]
            w3_sb = wmlp_sb[0:65, 128 : 128 + NOUT]

            nc.vector.tensor_copy(s_sb[0:H, :], s_psum[:])

            h1_ps = sm_psum[0:64, 64:80]
            nc.tensor.matmul(h1_ps, w1_sb, s_sb[:], start=True, stop=True)
            nc.vector.tensor_scalar_max(h1_sb[0:64, :], h1_ps, 0.0)

            h2_ps = sm_psum[0:64, 96:112]
            nc.tensor.matmul(h2_ps, w2_sb, h1_sb[:], start=True, stop=True)
            nc.vector.tensor_scalar_max(h2_sb[0:64, :], h2_ps, 0.0)

            o_ps = sm_psum[0:NOUT, 128:144]
            nc.tensor.matmul(o_ps, w3_sb, h2_sb[:], start=True, stop=True)
            o_sb = mlp_pool.tile([NOUT, BPC], FP32)
            nc.vector.tensor_copy(o_sb[:], o_ps)

            nc.sync.dma_start(out[:], o_sb[:])

    nc.compile()
    return nc


def _prep_shared(Wc, w1, b1, w2, b2, w3, b3):
    # reorder Wc columns: k = e*H + h  ->  k' = h*E + e
    Wc = np.asarray(Wc, dtype=np.float32)
    wc_r = np.ascontiguousarray(
        Wc.reshape(D, E, H).transpose(0, 2, 1).reshape(D, HE)
    )
    wc_stack = np.ascontiguousarray(
        np.concatenate([wc_r, wc_r], axis=0).astype(ml_dtypes.bfloat16)
    )
    wmlp = np.zeros((65, 138), np.float32)
    wmlp[0:H, 0:64] = np.asarray(w1, np.float32)
    wmlp[H, 0:64] = np.asarray(b1, np.float32)
    wmlp[0:64, 64:128] = np.asarray(w2, np.float32)
    wmlp[64, 64:128] = np.asarray(b2, np.float32)
    wmlp[0:64, 128 : 128 + NOUT] = np.asarray(w3, np.float32)
    wmlp[64, 128 : 128 + NOUT] = np.asarray(b3, np.float32)
    return dict(wc=wc_stack, wmlp=wmlp)


def _pack_x(Xc):
    # Xc [BPC, P, D] -> A [128, R//2]: A[64*(r%2)+d, r//2] = Xc_flat[r, d]
    Xf = np.asarray(Xc, np.float32).reshape(R, D)
    A = Xf.reshape(R // 2, 2, D).transpose(1, 2, 0).reshape(128, R // 2)
    return np.ascontiguousarray(A.astype(ml_dtypes.bfloat16))


def run(X, Wc, w1, b1, w2, b2, w3, b3, trace=False):
    if "nc" not in _cache:
        _cache["nc"] = _build_nc()
    nc = _cache["nc"]

    shared = _prep_shared(Wc, w1, b1, w2, b2, w3, b3)
    in_maps = []
    for c in range(NCORES):
        m = dict(shared)
        m["xa"] = _pack_x(X[c * BPC : (c + 1) * BPC])
        in_maps.append(m)

    res = run_bass_kernel_spmd(
        nc, in_maps, core_ids=list(range(NCORES)), trace=trace
    )
    outs = [np.asarray(r["out"]).T for r in res.results]  # each [BPC, NOUT]
    full = np.concatenate(outs, axis=0).astype(np.float32)
    return full, res


def kernel(X, Wc, w1, b1, w2, b2, w3, b3):
    full, _ = run(X, Wc, w1, b1, w2, b2, w3, b3, trace=False)
    return full

